# revision 16
# baseline (speedup 1.0000x reference)
"""Causal attention with clipped softmax on 8 TRN2 NeuronCores.

Problem: S=4096, H=16, D=128, B=1, fp32 inputs.
  scores = Q K^T / sqrt(D), causal mask, softmax,
  probs = clip(1.03*softmax - 0.03, 0, 1)   (== relu since upper clip never binds)
  out = probs @ V

Sharding: 2 heads per core (tensor parallel over heads), no collectives.

Per-core kernel (per head, per 128-row q-tile i, kv = 128*(i+1)):
  1. QK^T in bf16 (host-cast; PE full rate): psum scores [q=128, chunk<=1024],
     diagonal 128 cols get -1e9 upper-triangle accumulated via a second
     matmul (ident.T @ mask == mask) so no separate masking pass is needed
  2. one ACT Exp per chunk, psum->SBUF bf16; accum_out gives row sums Z free
  3. clip folded: probs = relu(1.03*e/Z - 0.03) is computed as
     t = relu(e - (0.03/1.03) Z) on DVE (tensor_scalar sub+max, split per
     transpose group).  V is pre-scaled by 1.03 on the host so the final
     row scale is a plain 1/Z (per-partition tensor_scalar_mul).
     GPSIMD is deliberately NOT used: measured ~10x slower than DVE here.
  4. PE transpose of t blocks (bf16, via identity) -> psum, DVE copyback
     in TGROUP=8 batches (fewer DVE instructions -> ~24us less DVE busy)
  5. PV: out[q,d] += tT_kb.T @ V_kb accumulated in psum (bf16 matmuls)
  6. software pipeline: stage_a (QK+exp) runs LOOKAHEAD tiles ahead of
     stage_b (relu/transpose/PV); the PV emission trails the transpose
     stream by PEND_DEPTH groups GLOBALLY (across tiles and heads), so
     the in-order PE never waits on the DVE copyback round trip and the
     psum output tile of tile i drains while tile i+1 is transposing.

TimelineSim (cost model): 194 us; engines busy PE 173 / ACT 173 / DVE 155,
so the kernel is within ~12% of the three-way engine-balance floor of this
dense design.

Measured (8 cores, axon/PJRT, REPS=8193 hardware-loop delta, median of 5,
run-to-run spread ~2%): 207.3 us/iter.  Input DMAs are issued in 1024-col
(qt/kt) and 8-block (v) chunks so the first QK/PV of each head depends on
a fraction of the transfer -- worth ~25 us/iter at head/loop boundaries.
The session-start baseline measures 277 us under the same methodology (its
published 239 us came from a noisy 1025-rep delta with ~40% spread).
"""

import math

import numpy as np
import ml_dtypes

S = 4096
H = 16
D = 128
N_CORES = 8
HPC = H // N_CORES  # heads per core
NQT = S // 128  # 32 q-tiles per head
SCALE = 1.0 / math.sqrt(D)
GAMMA = -0.03
ZETA = 1.0
A = ZETA - GAMMA  # 1.03
CHUNK = 1024  # scores chunk width (psum tile: 2 banks)
TGROUP = 8  # transpose blocks batched per psum tile / copyback
LOOKAHEAD = 4  # software pipeline depth (stage_a runs this far ahead)
EPOOL_BUFS = 5
PS_S_BUFS = 2
PS_T_BUFS = 2
PS_O_BUFS = 2
TT_BUFS = 5
PEND_DEPTH = 2
REPS = 1  # repeat whole kernel body (timing measurements only)

_CACHE = {}


def _build():
    import concourse.bass as bass  # noqa: F401
    import concourse.mybir as mybir
    import concourse.tile as tile
    from concourse import bacc
    from concourse.masks import make_identity

    dt = mybir.dt
    f32 = dt.float32
    bf16 = dt.bfloat16

    nc = bacc.Bacc("TRN2", target_bir_lowering=False, debug=False, num_devices=N_CORES)

    qt_d = nc.dram_tensor("qt", [HPC, 128, S], bf16, kind="ExternalInput")
    kt_d = nc.dram_tensor("kt", [HPC, 128, S], bf16, kind="ExternalInput")
    v_d = nc.dram_tensor("v", [HPC, 128, NQT, 128], bf16, kind="ExternalInput")
    o_d = nc.dram_tensor("o", [HPC, S, D], f32, kind="ExternalOutput")

    with tile.TileContext(nc) as tc:
        with (
            tc.tile_pool(name="const", bufs=1) as constp,
            tc.tile_pool(name="qk", bufs=2) as qkpool,
            tc.tile_pool(name="vp", bufs=2) as vpool,
            tc.tile_pool(name="ep", bufs=EPOOL_BUFS) as epool,
            tc.tile_pool(name="tp", bufs=2) as tpool,
            tc.tile_pool(name="ttp", bufs=TT_BUFS) as ttpool,
            tc.tile_pool(name="zp", bufs=EPOOL_BUFS + 1) as zpool,
            tc.tile_pool(name="op", bufs=3) as opool,
            tc.tile_pool(name="ps_s", bufs=PS_S_BUFS, space="PSUM") as ps_s,
            tc.tile_pool(name="ps_t", bufs=PS_T_BUFS, space="PSUM") as ps_t,
            tc.tile_pool(name="ps_o", bufs=PS_O_BUFS, space="PSUM") as ps_o,
        ):
            ident = constp.tile([128, 128], bf16)
            make_identity(nc, ident[:])
            # additive causal mask for the diagonal 128x128 block:
            # mbig[x, y] = 0.0 if x >= y else -1e9.  Accumulated into the
            # scores psum via matmul(lhsT=ident, rhs=mbig) => += mbig.
            mbig = constp.tile([128, 128], bf16)
            nc.gpsimd.memset(mbig[:], 0.0)
            nc.gpsimd.affine_select(
                out=mbig[:],
                in_=mbig[:],
                compare_op=mybir.AluOpType.is_ge,
                fill=-1e9,
                base=0,
                pattern=[[-1, 128]],
                channel_multiplier=1,
            )

            import contextlib
            rep_ctx = tc.For_i(0, REPS, 1) if REPS > 1 else contextlib.nullcontext()
            with rep_ctx:
                state = {}
                head_sb = {}  # h -> (qt_sb, kt_sb, v_sb)
                # pend: one (PV-group + final-tile bookkeeping) deferred
                # GLOBALLY across tiles and heads so the in-order PE always
                # has transpose work queued between a copyback and the PV
                # that consumes it.  Holding the last group's PV (and the
                # tile's osb/DMA) until the next tile's first transposes are
                # emitted removes a ~0.5us PE stall per tile.
                pend = []  # [(kb, g, tts, ops, nkb, v_sb, ascale, h, i), ...]

                def flush_pend():
                    if not pend:
                        return
                    kb, g, tts, ops, nkb, v_sb, ascale, h, i = pend.pop(0)
                    for j in range(g):
                        nc.tensor.matmul(
                            ops[:],
                            tts[:, j * 128 : (j + 1) * 128],
                            v_sb[:, kb + j, :],
                            start=(kb + j == 0),
                            stop=(kb + j == nkb - 1),
                            skip_group_check=True,
                        )
                    if kb + g == nkb:  # last group of tile -> finalize
                        osb = opool.tile([128, D], f32, tag="osb")
                        nc.vector.tensor_scalar_mul(osb[:], ops[:], ascale[:])
                        nc.sync.dma_start(
                            o_d.ap()[h, i * 128 : (i + 1) * 128, :], osb[:]
                        )

                def load_head(h):
                    qt_sb = qkpool.tile([128, S], bf16, tag="qt")
                    kt_sb = qkpool.tile([128, S], bf16, tag="kt")
                    v_sb = vpool.tile([128, NQT, 128], bf16, tag="v")
                    kchunk = min(1024, S)
                    for kc in range(S // kchunk):
                        # interleave qt/kt chunks so the first QK matmul
                        # (needs qt cols 0:128 + kt cols 0:128) depends on
                        # ~0.7us of DMA, not the full 2.9us transfers
                        nc.sync.dma_start(
                            qt_sb[:, kc * kchunk : (kc + 1) * kchunk],
                            qt_d.ap()[h, :, kc * kchunk : (kc + 1) * kchunk],
                        )
                        nc.sync.dma_start(
                            kt_sb[:, kc * kchunk : (kc + 1) * kchunk],
                            kt_d.ap()[h, :, kc * kchunk : (kc + 1) * kchunk],
                        )
                    for vc in range(0, NQT, 8):
                        nc.sync.dma_start(
                            v_sb[:, vc : vc + 8, :], v_d.ap()[h, :, vc : vc + 8, :]
                        )
                    head_sb[h] = (qt_sb, kt_sb, v_sb)

                def stage_a(h, i):
                    if h not in head_sb:
                        load_head(h)
                    # prefetch the next head's inputs mid-head: DMA queues
                    # are idle here, and issuing now removes the head0->
                    # head1 boundary wait on 3 MB of transfers
                    if i == 16 and (h + 1) < HPC and (h + 1) not in head_sb:
                        load_head(h + 1)
                    qt_sb, kt_sb, _ = head_sb[h]
                    kv = 128 * (i + 1)
                    e = epool.tile([128, S], bf16, tag="e")
                    zp = zpool.tile([128, 8], f32, tag="zpart")
                    qslice = qt_sb[:, i * 128 : (i + 1) * 128]
                    ncol = 0  # accum columns used
                    c0 = 0
                    while c0 < kv:
                        cn = min(CHUNK, kv - c0)
                        last_chunk = c0 + cn == kv
                        ps = ps_s.tile([128, CHUNK], f32, tag="s")
                        # QK^T chunk: matmuls of <=512 cols into one psum tile
                        m0 = 0
                        while m0 < cn:
                            mn = min(512, cn - m0)
                            has_diag = last_chunk and m0 + mn == cn
                            nc.tensor.matmul(
                                ps[:, m0 : m0 + mn],
                                qslice,
                                kt_sb[:, c0 + m0 : c0 + m0 + mn],
                                start=True,
                                stop=not has_diag,
                                skip_group_check=True,
                            )
                            m0 += mn
                        # accumulate -1e9 upper-triangle onto the diagonal
                        # 128 cols: ident.T @ mbig == mbig
                        nc.tensor.matmul(
                            ps[:, cn - 128 : cn],
                            ident[:],
                            mbig[:],
                            start=False,
                            stop=True,
                            skip_group_check=True,
                        ) if last_chunk else None
                        nc.scalar.activation(
                            e[:, c0 : c0 + cn],
                            ps[:, :cn],
                            mybir.ActivationFunctionType.Exp,
                            scale=SCALE,
                            accum_out=zp[:, ncol : ncol + 1],
                        )
                        ncol += 1
                        c0 += cn
                    state[(h, i)] = (e, zp, ncol)

                def stage_b(h, i):
                    _, _, v_sb = head_sb[h]
                    e, zp, ncol = state.pop((h, i))
                    if ncol > 1:
                        zsum = zpool.tile([128, 1], f32, tag="zsum")
                        nc.vector.tensor_reduce(
                            zsum[:], zp[:, :ncol], axis=mybir.AxisListType.X,
                            op=mybir.AluOpType.add,
                        )
                        zsum_ap = zsum[:]
                    else:
                        zsum_ap = zp[:, 0:1]
                    cbias = zpool.tile([128, 1], f32, tag="cbias")
                    nc.vector.tensor_scalar_mul(cbias[:], zsum_ap, GAMMA / -A)
                    # V is pre-scaled by A on the host, so the final
                    # per-partition scale is just 1/Z.
                    ascale = zpool.tile([128, 1], f32, tag="ascale")
                    nc.vector.reciprocal(ascale[:], zsum_ap)
                    # t = relu(e - cbias), split per transpose-group so the
                    # first transposes start after ~512 cols of relu instead
                    # of the whole row.
                    t = tpool.tile([128, S], bf16, tag="t")
                    ops = ps_o.tile([128, 128], f32, tag="o")
                    nkb = i + 1
                    groups = []
                    kb = 0
                    while kb < nkb:
                        groups.append((kb, min(TGROUP, nkb - kb)))
                        kb += TGROUP

                    for gi, (kb, g) in enumerate(groups):
                        lo, w = kb * 128, g * 128
                        nc.vector.tensor_scalar(
                            out=t[:, lo : lo + w],
                            in0=e[:, lo : lo + w],
                            scalar1=cbias[:],
                            scalar2=0.0,
                            op0=mybir.AluOpType.subtract,
                            op1=mybir.AluOpType.max,
                        )
                        tps = ps_t.tile([128, TGROUP * 128], bf16, tag="tt")
                        for j in range(g):
                            nc.tensor.transpose(
                                tps[:, j * 128 : (j + 1) * 128],
                                t[:, (kb + j) * 128 : (kb + j + 1) * 128],
                                ident[:],
                            )
                        tts = ttpool.tile([128, TGROUP * 128], bf16, tag="tts")
                        nc.vector.tensor_copy(tts[:, : g * 128], tps[:, : g * 128])
                        if len(pend) >= PEND_DEPTH:
                            flush_pend()
                        pend.append((kb, g, tts, ops, nkb, v_sb, ascale, h, i))

                # software pipeline: keep PE busy during softmax of tile i
                tiles = [(h, i) for h in range(HPC) for i in range(NQT)]
                for idx in range(len(tiles) + LOOKAHEAD):
                    if idx < len(tiles):
                        stage_a(*tiles[idx])
                    if idx >= LOOKAHEAD:
                        stage_b(*tiles[idx - LOOKAHEAD])
                while pend:
                    flush_pend()

    nc.compile()
    return nc


def _get_nc():
    if "nc" not in _CACHE:
        _CACHE["nc"] = _build()
    return _CACHE["nc"]


def kernel(query_states, key_states, value_states, q_sequence_mask, kv_sequence_mask):
    from concourse import bass_utils

    nc = _get_nc()

    q = np.asarray(query_states, dtype=np.float32)
    k = np.asarray(key_states, dtype=np.float32)
    v = np.asarray(value_states, dtype=np.float32)

    in_maps = []
    for c in range(N_CORES):
        hs = slice(HPC * c, HPC * (c + 1))
        # [S, hpc, D] -> [hpc, D, S]
        qt = np.ascontiguousarray(q[:, hs, :].transpose(1, 2, 0)).astype(
            ml_dtypes.bfloat16
        )
        kt = np.ascontiguousarray(k[:, hs, :].transpose(1, 2, 0)).astype(
            ml_dtypes.bfloat16
        )
        # [S, hpc, D] -> [hpc, S, D] -> [hpc, kb, p, D] -> [hpc, p, kb, D]
        # pre-scaled by A so the on-device output scale is 1/Z
        vc = (
            (v[:, hs, :] * A)
            .transpose(1, 0, 2)
            .reshape(HPC, NQT, 128, D)
            .transpose(0, 2, 1, 3)
        )
        vc = np.ascontiguousarray(vc).astype(ml_dtypes.bfloat16)
        in_maps.append({"qt": qt, "kt": kt, "v": vc})

    res = bass_utils.run_bass_kernel_spmd(
        nc, in_maps, core_ids=list(range(N_CORES))
    )

    out = np.empty((S, H, D), dtype=np.float32)
    for c in range(N_CORES):
        oc = res.results[c]["o"]  # [hpc, S, D]
        for hh in range(HPC):
            out[:, HPC * c + hh, :] = oc[hh]
    return out

